# revision 1
# baseline (speedup 1.0000x reference)
"""Trainium2 Bass kernel for nn_Cell_First (gnn_message_passing).

Reference computation (see problem): a 3-node NAS cell over a graph.
  states = [h]; for i in 0..2: s_{i+1} = sum_j mixed(m_ij, states[j])
  mixed(m, x) = sum_c  w[m,c] * relu(BN(branch_c(x) @ W[m,c].T + b[m,c]))
  branches = (mean-neighbor-agg(x), x, h_in);  BN = batch norm over all N nodes.
Output: stack(s1, s2, s3)  [3, N, 128] f32.

Strategy (8 NeuronCores, SPMD):
 - Nodes sharded: core k owns dst nodes [k*6250, (k+1)*6250); edges partitioned
   by dst owner (host-side graph partitioning, per the sharding hint).
 - Aggregation: edges sorted by dst into 32-slot windows; per 128-edge tile the
   host builds a [128, 32] one-hot matrix (inv-degree folded in).  On device:
   dma_gather fetches x[src] rows (fp16, 256B each) from a replicated DRAM
   table, TensorE computes  G.T @ onehot  accumulating per-window in PSUM ->
   agg^T [128ch, nodes] directly.  int16 gather indices => tables split at
   row 32768 (lo/hi gather calls).
 - BatchNorm stats computed analytically from per-branch-input moments
   (C = x^T x, S = sum x), all-reduced across cores (tiny), then folded with
   the per-candidate weights into a per-channel scale/bias applied by ScalarE
   as relu(scale*z + bias) in one pass (z kept transposed [ch, nodes] so the
   affine is per-partition).
 - Cross-core exchange: AllGather of new state tables (fp16) per round;
   AllReduce of moment blocks per round.
 - Output produced channel-major per core; host transposes/concatenates.
"""

import numpy as np
import ml_dtypes

BF16 = np.float16  # fp16: 10 mantissa bits, full PE speed

N, D, E, NC = 50000, 128, 800000, 8
NLOC = N // NC            # 6250
WIN = 32                  # dst slots per window (one-hot width)
SPLIT = 32768             # int16 gather index limit
NWIN = (NLOC + WIN - 1) // WIN        # 196
WPB = 16                  # windows per 512-slot block
NBLK = (NWIN + WPB - 1) // WPB        # 13
NT_NODES = (NLOC + 127) // 128        # 49 node-major tiles
TAIL = NLOC - 128 * (NT_NODES - 1)    # 106
ZC = [512] * (NLOC // 512) + ([NLOC % 512] if NLOC % 512 else [])  # z chunks
EPS = 1e-5
ROUND_MS = [[0], [1, 2], [3, 4, 5]]   # mixed-op ids per round
# moment-state ids
S_H, S_HIN, S_S1, S_S2, S_A0, S_A1, S_A2 = range(7)
STATE_OF = {0: S_H, 1: S_S1, 2: S_S2}
AGG_OF = {0: S_A0, 1: S_A1, 2: S_A2}


# ---------------------------------------------------------------- host prep
def preprocess(edge_index):
    """Partition/sort edges, build per-core gather indices + one-hot rhs."""
    src = np.asarray(edge_index[0], dtype=np.int64)
    dst = np.asarray(edge_index[1], dtype=np.int64)
    deg = np.bincount(dst, minlength=N)
    invdeg = (1.0 / np.maximum(deg, 1.0)).astype(np.float32)

    core = dst // NLOC
    dstl = dst % NLOC
    win = dstl // WIN
    half = (src >= SPLIT).astype(np.int64)

    order = np.lexsort((dstl, half, win, core))
    s_src, s_dst, s_core, s_dstl, s_win, s_half = (
        src[order], dst[order], core[order], dstl[order], win[order], half[order])

    # rank within (core, win, half) group
    gid = (s_core * NWIN + s_win) * 2 + s_half
    counts = np.bincount(gid, minlength=NC * NWIN * 2)
    gstart = np.concatenate([[0], np.cumsum(counts)[:-1]])
    rank = np.arange(len(s_src)) - gstart[gid]

    cnt = counts.reshape(NC, NWIN, 2)
    tlo = np.maximum(1, np.ceil(cnt[:, :, 0].max(axis=0) / 128).astype(np.int64))
    thi = np.ceil(cnt[:, :, 1].max(axis=0) / 128).astype(np.int64)

    # global tile order: per block: lo tiles (w asc, k asc) then hi tiles
    tbase_lo = np.zeros(NWIN, np.int64)
    tbase_hi = np.zeros(NWIN, np.int64)
    calls = []  # per block: (t0_lo, nt_lo, t0_hi, nt_hi)
    t = 0
    for b in range(NBLK):
        ws = range(b * WPB, min((b + 1) * WPB, NWIN))
        t0_lo = t
        for w in ws:
            tbase_lo[w] = t
            t += tlo[w]
        t0_hi = t
        for w in ws:
            tbase_hi[w] = t
            t += thi[w]
        calls.append((t0_lo, t0_hi - t0_lo, t0_hi, t - t0_hi))
    nt_total = t

    tile_of = np.where(s_half == 0, tbase_lo[s_win], tbase_hi[s_win]) + rank // 128
    part_of = rank % 128

    per_core = []
    for c in range(NC):
        m = s_core == c
        tiles_c, parts_c = tile_of[m], part_of[m]
        idxflat = np.zeros(nt_total * 128, np.int32)
        idxval = np.where(s_half[m] == 0, s_src[m], s_src[m] - SPLIT)
        idxflat[tiles_c * 128 + parts_c] = idxval
        idxw = np.zeros((16, nt_total * 8), np.int16)
        fl = np.arange(nt_total * 128)
        idxw[fl % 16, fl // 16] = idxflat.astype(np.int16)
        idxw = np.tile(idxw, (8, 1))  # [128, nt*8]

        rhs = np.zeros((nt_total, 128, WIN), np.float32)
        cols_c = s_dstl[m] - WIN * s_win[m]
        rhs[tiles_c, parts_c, cols_c] = invdeg[s_dst[m]]
        rhs = np.ascontiguousarray(
            rhs.transpose(1, 0, 2).reshape(128, nt_total * WIN)).astype(BF16)
        per_core.append((idxw, rhs))

    return dict(tlo=tlo, thi=thi, calls=calls, nt=nt_total,
                per_core=per_core, invdeg=invdeg)


def make_host_inputs(h, h_in, weights, W, b, gamma, beta):
    """Small replicated tensors + per-core state slices."""
    h = np.asarray(h, np.float32)
    h_in = np.asarray(h_in, np.float32)
    table_h = h.astype(BF16)                                    # [N, 128]
    wT = np.stack([W[m, c].T for m in range(6) for c in range(3)])  # [18,128,128]
    wT = np.ascontiguousarray(
        wT.transpose(1, 0, 2).reshape(128, 18 * 128)).astype(BF16)
    # bn_small [128, 54] f32: cols 3*mc + {0:wgam, 1:wbeta, 2:blin}
    bn = np.zeros((128, 54), np.float32)
    for m in range(6):
        for c in range(3):
            mc = m * 3 + c
            bn[:, 3 * mc + 0] = weights[m, c] * gamma[m, c]
            bn[:, 3 * mc + 1] = weights[m, c] * beta[m, c]
            bn[:, 3 * mc + 2] = b[m, c]
    per_core = []
    for k in range(NC):
        sl = slice(k * NLOC, (k + 1) * NLOC)
        per_core.append(dict(
            hT=np.ascontiguousarray(h[sl].T).astype(BF16),       # [128, 6250]
            hinT=np.ascontiguousarray(h_in[sl].T).astype(BF16),
            hin_nm=h_in[sl].astype(BF16),                        # [6250, 128]
        ))
    return table_h, wT, bn, per_core


# ---------------------------------------------------------------- device build
def build(struct, stage=99, iters=1):
    """stage: 1=agg0 only, 2=+moments/AR/bn, 3=+z0, 4=+finalize/AG/agg1,
    99=full"""
    import concourse.bass as bass
    import concourse.bacc as bacc
    import concourse.tile as tile
    import concourse.mybir as mybir
    from concourse.masks import make_identity

    dt = mybir.dt
    AF = mybir.ActivationFunctionType
    OP = mybir.AluOpType
    NT = struct["nt"]
    tlo, thi, calls = struct["tlo"], struct["thi"], struct["calls"]

    nc = bacc.Bacc("TRN2", target_bir_lowering=False, debug=False)

    table_h = nc.dram_tensor("table_h", [N, D], dt.float16, kind="ExternalInput")
    idxs_in = nc.dram_tensor("idxs", [128, NT * 8], dt.int16, kind="ExternalInput")
    rhs_in = nc.dram_tensor("rhs", [128, NT * WIN], dt.float16, kind="ExternalInput")
    wT_in = nc.dram_tensor("wT", [128, 18 * 128], dt.float16, kind="ExternalInput")
    bn_in = nc.dram_tensor("bn_small", [128, 54], dt.float32, kind="ExternalInput")
    hT_in = nc.dram_tensor("hT", [128, NLOC], dt.float16, kind="ExternalInput")
    hinT_in = nc.dram_tensor("hinT", [128, NLOC], dt.float16, kind="ExternalInput")
    hin_nm_in = nc.dram_tensor("hin_nm", [NLOC, D], dt.float16, kind="ExternalInput")
    out_cm = nc.dram_tensor("out_cm", [3, 128, NLOC], dt.float32, kind="ExternalOutput")

    ARW = 129 * 3  # allreduce width (3 moment blocks max)

    with tile.TileContext(nc) as tc:
        import contextlib
        ctx = contextlib.ExitStack()
        with ctx:
            cst = ctx.enter_context(tc.tile_pool(name="cst", bufs=1))
            gat_p = ctx.enter_context(tc.tile_pool(name="gat", bufs=3))
            rhs_p = ctx.enter_context(tc.tile_pool(name="rhsp", bufs=2))
            ztmp_p = ctx.enter_context(tc.tile_pool(name="ztmp", bufs=2))
            nmt_p = ctx.enter_context(tc.tile_pool(name="nmt", bufs=3))
            sv_p = ctx.enter_context(tc.tile_pool(name="sv", bufs=2))
            agg_ps = ctx.enter_context(tc.tile_pool(name="aggps", bufs=2, space="PSUM"))
            z_ps = ctx.enter_context(tc.tile_pool(name="zps", bufs=2, space="PSUM"))
            sm_ps = ctx.enter_context(tc.tile_pool(name="smps", bufs=2, space="PSUM"))
            dram = ctx.enter_context(tc.tile_pool(name="dram", bufs=1, space="DRAM"))

            # ---------- resident tiles (packed to avoid 4KB/tile padding) ----
            idx_sb = cst.tile([128, NT * 8], dt.int16)
            wt_sb = cst.tile([128, 18 * 128], dt.float16)
            xt_all = cst.tile([128, 4 * NLOC], dt.float16)
            hT = xt_all[:, 0 * NLOC:1 * NLOC]
            hinT = xt_all[:, 1 * NLOC:2 * NLOC]
            s1T = xt_all[:, 2 * NLOC:3 * NLOC]
            s2T = xt_all[:, 3 * NLOC:4 * NLOC]
            agg_all = cst.tile([128, 3 * NLOC], dt.float16)
            aggT = [agg_all[:, a * NLOC:(a + 1) * NLOC] for a in range(3)]
            acc = cst.tile([128, NLOC], dt.float32)
            # packed bf16 small buffer: ident(128) c_bf(7x128) s_bf(7) ones(1)
            sb_bf = cst.tile([128, 128 + 7 * 128 + 7 + 1], dt.float16)
            ident = sb_bf[:, 0:128]
            c_bf = [sb_bf[:, 128 + 128 * s:128 + 128 * (s + 1)] for s in range(7)]
            s_bf = [sb_bf[:, 1024 + s:1025 + s] for s in range(7)]
            ones_bf = sb_bf[:, 1031:1032]
            # packed f32 small buffer: bn(54) arstage(387) ar_sb(387) scale(18)
            # bias(18) eps(1)
            sb_f32 = cst.tile([128, 54 + 2 * ARW + 18 + 18 + 1], dt.float32)
            bn_sb = sb_f32[:, 0:54]
            arstage = sb_f32[:, 54:54 + ARW]
            ar_sb = sb_f32[:, 54 + ARW:54 + 2 * ARW]
            bn_scale = sb_f32[:, 54 + 2 * ARW:54 + 2 * ARW + 18]
            bn_bias = sb_f32[:, 54 + 2 * ARW + 18:54 + 2 * ARW + 36]
            eps_sv = sb_f32[:, 54 + 2 * ARW + 36:54 + 2 * ARW + 37]

            ag_in = dram.tile([NLOC, D], dt.float16)
            ar_ins = [dram.tile([128, ARW], dt.float32, name=f"ar_in{r}")
                      for r in range(3)]
            cur = {}  # per-iteration Shared collective buffers

            # ---------- prep ----------
            nc.sync.dma_start(idx_sb[:], idxs_in[:])
            nc.sync.dma_start(wt_sb[:], wT_in[:])
            nc.sync.dma_start(bn_sb[:], bn_in[:])
            nc.sync.dma_start(hT[:], hT_in[:])
            nc.sync.dma_start(hinT[:], hinT_in[:])
            nc.gpsimd.memset(ones_bf[:], 1.0)
            nc.gpsimd.memset(eps_sv[:], EPS)
            make_identity(nc, ident[:])

            def moment_psums():
                mt = sm_ps.tile([128, 129], dt.float32, space="PSUM", tag="momCS",
                                bufs=1)
                return mt[:, 0:128], mt[:, 128:129]

            def moment_acc(cps, sps, xt, t):
                # start=True only on the very first matmul of the shared bank
                # (it clears the whole bank; S's first write lands on
                # has_written=0 and overwrites).
                nc.tensor.matmul(cps, lhsT=xt, rhs=xt,
                                 start=(t == 0), stop=(t == NT_NODES - 1),
                                 skip_group_check=True)
                nc.tensor.matmul(sps, lhsT=xt, rhs=ones_bf[:],
                                 start=False, stop=(t == NT_NODES - 1),
                                 skip_group_check=True)

            def moment_evict(cps, sps, blk):
                co = 129 * blk
                nc.vector.tensor_scalar_mul(arstage[:, co:co + 128], cps[:], 1.0 / N)
                nc.vector.tensor_scalar_mul(arstage[:, co + 128:co + 129], sps[:],
                                            1.0 / N)

            def moments_from_dram(dram_rows, blk):
                """Stream node-major DRAM rows tile-by-tile into moment psums."""
                cps, sps = moment_psums()
                for t in range(NT_NODES):
                    wv = min(128, NLOC - 128 * t)
                    nm_t = nmt_p.tile([128, 128], dt.float16, tag="nmt")
                    if wv < 128:
                        nc.gpsimd.memset(nm_t[96:, :], 0)
                    nc.sync.dma_start(nm_t[:wv, :],
                                      dram_rows[128 * t:128 * t + wv, :])
                    moment_acc(cps, sps, nm_t[:], t)
                moment_evict(cps, sps, blk)

            # h, h_in moments (blocks 0, 1 of round-0 AR)
            moments_from_dram(table_h[:], 0)
            moments_from_dram(hin_nm_in[:], 1)

            # ---------- aggregation ----------
            def agg_phase(a, table):
                dst = aggT[a]
                momC, momS = moment_psums()
                tn_all = 0
                GCH = 32  # gather chunk (tiles per call); needs single_packet=False
                for b in range(NBLK):
                    ws = list(range(b * WPB, min((b + 1) * WPB, NWIN)))
                    t0_lo, nt_lo, t0_hi, nt_hi = calls[b]
                    nt_b = nt_lo + nt_hi  # tiles [t0_lo, t0_lo + nt_b) contiguous
                    rhs_t = rhs_p.tile([128, nt_b * WIN], dt.float16, tag="rhsb")
                    nc.sync.dma_start(
                        rhs_t[:], rhs_in[:, WIN * t0_lo:WIN * (t0_lo + nt_b)])

                    def rhs_ap(t, _r=rhs_t, _t0=t0_lo):
                        return _r[:, WIN * (t - _t0):WIN * (t - _t0 + 1)]

                    # chunked gathers; tile_ap[t] -> lhsT access for global tile t
                    tile_ap = {}
                    for (t0, nt, tab) in ((t0_lo, nt_lo, table[:]),
                                          (t0_hi, nt_hi, table[SPLIT:, :])):
                        for c0 in range(t0, t0 + nt, GCH):
                            ct = min(GCH, t0 + nt - c0)
                            g = gat_p.tile([128, ct, 128], dt.float16, tag="gat")
                            nc.gpsimd.dma_gather(
                                g[:], tab, idx_sb[:, 8 * c0:8 * (c0 + ct)],
                                ct * 128, ct * 128, 128, single_packet=False)
                            for i in range(ct):
                                tile_ap[c0 + i] = g[:, i, :]
                    # NOTE: start=True clears the WHOLE psum bank, so only the
                    # block's first matmul uses it; per-element has_written
                    # bits make later first-writes overwrite / repeats add.
                    P = agg_ps.tile([128, 512], dt.float32, space="PSUM", tag="P")
                    last_t = t0_lo + nt_b - 1
                    t = t0_lo
                    for w in ws:
                        co = 32 * (w - b * WPB)
                        for k in range(tlo[w]):
                            nc.tensor.matmul(
                                P[:, co:co + 32], lhsT=tile_ap[t], rhs=rhs_ap(t),
                                start=(t == t0_lo), stop=(t == last_t),
                                skip_group_check=True)
                            t += 1
                    t = t0_hi
                    for w in ws:
                        co = 32 * (w - b * WPB)
                        for k in range(thi[w]):
                            nc.tensor.matmul(
                                P[:, co:co + 32], lhsT=tile_ap[t], rhs=rhs_ap(t),
                                start=False, stop=(t == last_t),
                                skip_group_check=True)
                            t += 1
                    nbc = min(512, NLOC - 512 * b)
                    nc.vector.tensor_copy(dst[:, 512 * b:512 * b + nbc], P[:, :nbc])
                    # transpose freshly evicted node tiles + accumulate moments
                    while tn_all * 128 < 512 * b + nbc:
                        tn = tn_all
                        wv = min(128, NLOC - 128 * tn)
                        tp = sm_ps.tile([128, 128], dt.float16, space="PSUM", tag="tp")
                        nc.tensor.transpose(tp[:wv, :], dst[:, 128 * tn:128 * tn + wv],
                                            ident[:])
                        nm_t = nmt_p.tile([128, 128], dt.float16, tag="nmt")
                        if wv < 128:
                            nc.gpsimd.memset(nm_t[96:, :], 0)
                        nc.vector.tensor_copy(nm_t[:wv, :], tp[:wv, :])
                        moment_acc(momC, momS, nm_t[:], tn)
                        tn_all += 1
                moment_evict(momC, momS, 2 if a == 0 else 1)

            # ---------- allreduce + unpack ----------
            def allreduce_round(r, blocks):
                """blocks: list of (arstage block idx, state id)"""
                ar_in, ar_out = ar_ins[r], cur["ar_outs"][r]
                nc.gpsimd.dma_start(ar_in[:], arstage[:])
                nc.gpsimd.collective_compute(
                    "AllReduce", OP.add, replica_groups=[list(range(NC))],
                    ins=[ar_in.opt()], outs=[ar_out.opt()])
                nc.gpsimd.dma_start(ar_sb[:], ar_out[:])
                for blk, sid in blocks:
                    co = 129 * blk
                    nc.vector.tensor_copy(c_bf[sid][:], ar_sb[:, co:co + 128])
                    nc.vector.tensor_copy(s_bf[sid][:], ar_sb[:, co + 128:co + 129])

            # ---------- BN params for one (m, c) ----------
            def bn_params(mc, sid):
                wt = wt_sb[:, 128 * mc:128 * (mc + 1)]
                wgam = bn_sb[:, 3 * mc + 0:3 * mc + 1]
                wbeta = bn_sb[:, 3 * mc + 1:3 * mc + 2]
                blin = bn_sb[:, 3 * mc + 2:3 * mc + 3]
                vps = sm_ps.tile([128, 128], dt.float32, space="PSUM", tag="tp")
                nc.tensor.matmul(vps[:], lhsT=c_bf[sid][:], rhs=wt, start=True, stop=True)
                vsb = nmt_p.tile([128, 128], dt.float16, tag="nmt")
                nc.vector.tensor_copy(vsb[:], vps[:])
                msb = nmt_p.tile([128, 128], dt.float16, tag="nmt")
                nc.vector.tensor_tensor(out=msb[:], in0=vsb[:], in1=wt, op=OP.mult)
                bnv = sm_ps.tile([128, 2], dt.float32, space="PSUM", tag="bnv", bufs=1)
                d2, ws_ = bnv[:, 0:1], bnv[:, 1:2]
                nc.tensor.matmul(d2, lhsT=msb[:], rhs=ones_bf[:], start=True, stop=True,
                                 skip_group_check=True)
                nc.tensor.matmul(ws_, lhsT=wt, rhs=s_bf[sid][:], start=False, stop=True,
                                 skip_group_check=True)
                sv = sv_p.tile([128, 10], dt.float32, tag="sv")
                mu, t1, t2, e2, mu2, var, sd, rv, t4, t5 = (
                    sv[:, i:i + 1] for i in range(10))
                nc.vector.tensor_tensor(out=mu, in0=ws_[:], in1=blin, op=OP.add)
                nc.vector.tensor_scalar(out=t1, in0=ws_[:], scalar1=2.0,
                                        scalar2=blin, op0=OP.mult, op1=OP.add)
                nc.vector.tensor_tensor(out=t2, in0=t1, in1=blin, op=OP.mult)
                nc.vector.tensor_tensor(out=e2, in0=d2[:], in1=t2, op=OP.add)
                nc.vector.tensor_tensor(out=mu2, in0=mu, in1=mu, op=OP.mult)
                nc.vector.tensor_tensor(out=var, in0=e2, in1=mu2, op=OP.subtract)
                nc.scalar.activation(sd, var, AF.Sqrt, bias=eps_sv, scale=1.0)
                nc.vector.reciprocal(rv, sd)
                nc.vector.tensor_tensor(out=bn_scale[:, mc:mc + 1], in0=wgam, in1=rv,
                                        op=OP.mult)
                nc.vector.tensor_tensor(out=t4, in0=blin, in1=mu, op=OP.subtract)
                nc.vector.tensor_tensor(out=t5, in0=bn_scale[:, mc:mc + 1], in1=t4,
                                        op=OP.mult)
                nc.vector.tensor_tensor(out=bn_bias[:, mc:mc + 1], in0=wbeta, in1=t5,
                                        op=OP.add)

            # ---------- z phase ----------
            def z_phase(r):
                first = True
                for j, m in enumerate(ROUND_MS[r]):
                    for c in range(3):
                        mc = m * 3 + c
                        rx = (aggT[j] if c == 0 else
                              [hT, s1T, s2T][j] if c == 1 else hinT)
                        lw = wt_sb[:, 128 * mc:128 * (mc + 1)]
                        off = 0
                        for cw in ZC:
                            zp = z_ps.tile([128, 512], dt.float32, space="PSUM", tag="zp")
                            nc.tensor.matmul(zp[:, :cw], lhsT=lw,
                                             rhs=rx[:, off:off + cw],
                                             start=True, stop=True)
                            if first:
                                nc.scalar.activation(
                                    acc[:, off:off + cw], zp[:, :cw], AF.Relu,
                                    bias=bn_bias[:, mc:mc + 1],
                                    scale=bn_scale[:, mc:mc + 1])
                            else:
                                zt = ztmp_p.tile([128, 512], dt.float32, tag="zt")
                                nc.scalar.activation(
                                    zt[:, :cw], zp[:, :cw], AF.Relu,
                                    bias=bn_bias[:, mc:mc + 1],
                                    scale=bn_scale[:, mc:mc + 1])
                                nc.vector.tensor_tensor(
                                    out=acc[:, off:off + cw], in0=acc[:, off:off + cw],
                                    in1=zt[:, :cw], op=OP.add)
                            off += cw
                        first = False

            # ---------- finalize state r (s1/s2): cast, nm transpose, moments, AG
            def finalize_state(r):
                sT = [s1T, s2T][r]
                off = 0
                for cw in ZC:
                    nc.vector.tensor_copy(sT[:, off:off + cw], acc[:, off:off + cw])
                    off += cw
                cps, sps = moment_psums()
                for tn in range(NT_NODES):
                    wv = min(128, NLOC - 128 * tn)
                    tp = sm_ps.tile([128, 128], dt.float16, space="PSUM", tag="tp")
                    nc.tensor.transpose(tp[:wv, :], sT[:, 128 * tn:128 * tn + wv],
                                        ident[:])
                    nm_t = nmt_p.tile([128, 128], dt.float16, tag="nmt")
                    if wv < 128:
                        nc.gpsimd.memset(nm_t[96:, :], 0)
                    nc.vector.tensor_copy(nm_t[:wv, :], tp[:wv, :])
                    moment_acc(cps, sps, nm_t[:], tn)
                    nc.gpsimd.dma_start(ag_in[128 * tn:128 * tn + wv, :],
                                        nm_t[:wv, :])
                moment_evict(cps, sps, 0)
                tbl = cur["tbls"][r]
                nc.gpsimd.collective_compute(
                    "AllGather", OP.bypass, replica_groups=[list(range(NC))],
                    ins=[ag_in.opt()], outs=[tbl.opt()])
                return tbl

            # ================= main schedule =================
            def emit():
              agg_phase(0, table_h)
              if stage == 1:
                off = 0
                for cw in ZC:
                    nc.vector.tensor_copy(acc[:, off:off + cw],
                                          aggT[0][:, off:off + cw])
                    off += cw
                nc.sync.dma_start(out_cm[0], acc[:])
                return
              allreduce_round(0, [(0, S_H), (1, S_HIN), (2, S_A0)])
              for c in range(3):
                bn_params(0 * 3 + c, [S_A0, S_H, S_HIN][c])
              if stage == 2:
                nc.vector.tensor_copy(acc[:, 0:18], bn_scale[:])
                nc.vector.tensor_copy(acc[:, 18:36], bn_bias[:])
                nc.sync.dma_start(out_cm[0], acc[:])
                return
              z_phase(0)
              nc.sync.dma_start(out_cm[0], acc[:])
              if stage == 3:
                return
              tbl1 = finalize_state(0)

              agg_phase(1, tbl1)
              if stage == 4:
                off = 0
                for cw in ZC:
                    nc.vector.tensor_copy(acc[:, off:off + cw],
                                          aggT[1][:, off:off + cw])
                    off += cw
                nc.sync.dma_start(out_cm[1], acc[:])
                return
              allreduce_round(1, [(0, S_S1), (1, S_A1)])
              for m in ROUND_MS[1]:
                for c in range(3):
                    sid = [AGG_OF[m - 1], STATE_OF[m - 1], S_HIN][c]
                    bn_params(m * 3 + c, sid)
              z_phase(1)
              nc.sync.dma_start(out_cm[1], acc[:])
              tbl2 = finalize_state(1)

              agg_phase(2, tbl2)
              allreduce_round(2, [(0, S_S2), (1, S_A2)])
              for m in ROUND_MS[2]:
                for c in range(3):
                    j = m - 3
                    sid = [AGG_OF[j], STATE_OF[j], S_HIN][c]
                    bn_params(m * 3 + c, sid)
              z_phase(2)
              nc.sync.dma_start(out_cm[2], acc[:])

            for it in range(iters):
                cur["tbls"] = [
                    dram.tile([N, D], dt.float16, addr_space="Shared",
                              name=f"tbl_s{r}_{it}") for r in range(2)]
                cur["ar_outs"] = [
                    dram.tile([128, ARW], dt.float32, addr_space="Shared",
                              name=f"ar_out{r}_{it}") for r in range(3)]
                emit()

    nc.compile()
    return nc


# ---------------------------------------------------------------- entry point
def prepare(edge_index, h, h_in, weights, W, b, gamma, beta):
    struct = preprocess(edge_index)
    table_h, wT, bn, per_core_s = make_host_inputs(h, h_in, weights, W, b, gamma, beta)
    in_maps = []
    for k in range(NC):
        idxw, rhs = struct["per_core"][k]
        in_maps.append(dict(
            table_h=table_h, idxs=idxw, rhs=rhs, wT=wT, bn_small=bn,
            hT=per_core_s[k]["hT"], hinT=per_core_s[k]["hinT"],
            hin_nm=per_core_s[k]["hin_nm"]))
    nc = build(struct)
    return nc, in_maps


def assemble(results):
    out = np.empty((3, N, D), np.float32)
    for k in range(NC):
        cm = results[k]["out_cm"]  # [3, 128, NLOC]
        for r in range(3):
            out[r, k * NLOC:(k + 1) * NLOC, :] = cm[r].T
    return out


def kernel(edge_index, h, h_in, weights, W, b, gamma, beta):
    from concourse.bass_utils import run_bass_kernel_spmd
    nc, in_maps = prepare(np.asarray(edge_index), h, h_in,
                          np.asarray(weights, np.float32),
                          np.asarray(W, np.float32), np.asarray(b, np.float32),
                          np.asarray(gamma, np.float32), np.asarray(beta, np.float32))
    res = run_bass_kernel_spmd(nc, in_maps, core_ids=list(range(NC)))
    return assemble(res.results)



# revision 13
# speedup vs baseline: 1.3144x; 1.3144x over previous
"""Trainium2 Bass kernel for nn_Cell_First (gnn_message_passing).

Reference: 3-node NAS cell over a graph (N=50000 nodes, E=800000 edges,
D=128).  states=[h]; s_{i+1} = sum_j mixed(m_ij, states[j]);
mixed(m,x) = sum_c w[m,c]*relu(BN(branch_c(x) @ W[m,c].T + b[m,c]));
branches = (mean-neighbor-agg(x), x, h_in).  Output stack(s1,s2,s3).

Distribution (8 cores): nodes sharded by dst; edges partitioned by dst
owner.  Aggregation via dma_gather of src rows from a replicated fp8
table + one-hot TensorE matmuls accumulating agg^T in PSUM.

Key layout tricks vs the v0 kernel:
 - fp8(e3m4) gather tables packed as [P/2, 256B] node-PAIR rows; gather
   payload = 128B at 256B stride (2x cheaper descriptors), idx = table
   position >> 1 (fits int16 without hi/lo split); edges grouped by
   position parity (even/odd table views).
 - per-window shared capacities w/o 128-rounding (padding only at group
   ends); a 128-edge tile spans windows, split matmuls at window bounds.
 - state tables split in 2 chunks, AllGathered separately so next-round
   gathers overlap the collective.
 - invdeg applied at PSUM eviction (one-hot rhs holds exact 1.0 in fp8).
 - AllReduces split: state moments (fired early, hidden under AG/gather)
   vs agg moments (fired at agg end, hidden under identity/skip-branch z
   matmuls).
"""

import numpy as np
import ml_dtypes

BF16 = np.float16              # fp16 for states/weights on device
FP8 = ml_dtypes.float8_e3m4    # gather tables / one-hot rhs

N, D, E, NC = 50000, 128, 800000, 8
NLOC = N // NC                 # 6250
WIN = 32                       # dst slots per window (one-hot width)
WPB = 16                       # windows per 512-slot block
NWIN = (NLOC + WIN - 1) // WIN         # 196
NBLK = (NWIN + WPB - 1) // WPB         # 13
NT_NODES = (NLOC + 127) // 128         # 49 node-major tiles
CHL = 3072                     # local rows in table chunk 0 (tile-aligned)
CH0 = NC * CHL                 # chunk-0 table rows (24576)
CH1 = N - CH0                  # chunk-1 table rows (25424)
ZC = [512] * (NLOC // 512) + ([NLOC % 512] if NLOC % 512 else [])
EPS = 1e-5
ROUND_MS = [[0], [1, 2], [3, 4, 5]]
# moment-state ids
S_H, S_HIN, S_S1, S_S2, S_A0, S_A1, S_A2 = range(7)
GCH = 32                       # gather chunk (tiles per dma_gather call)


def _pos_of(node):
    """Table position of a node: chunk0 = per-core local rows [0,CHL),
    chunk1 = the rest, concatenated per core (AllGather layouts)."""
    k = node // NLOC
    l = node % NLOC
    return np.where(l < CHL, CHL * k + l, CH0 + (NLOC - CHL) * k + (l - CHL))


# ---------------------------------------------------------------- host prep
def preprocess(edge_index):
    """Partition/sort edges; build shared tile/run structure + per-core
    gather indices and one-hot rhs."""
    src = np.asarray(edge_index[0], dtype=np.int64)
    dst = np.asarray(edge_index[1], dtype=np.int64)
    deg = np.bincount(dst, minlength=N)
    invdeg = (1.0 / np.maximum(deg, 1.0)).astype(np.float32)

    core = dst // NLOC
    dstl = dst % NLOC
    win = dstl // WIN                     # global window 0..NWIN-1
    pos = _pos_of(src)                    # table position
    q = pos & 1                           # position parity
    ch = (src % NLOC >= CHL).astype(np.int64)   # table chunk

    # group = (win, q, ch); per-core counts -> shared caps
    gid3 = (win * 2 + q) * 2 + ch
    cgid = core * (NWIN * 4) + gid3
    cnt = np.bincount(cgid, minlength=NC * NWIN * 4).reshape(NC, NWIN, 2, 2)
    cap = cnt.max(axis=0)                 # [NWIN, 2, 2]

    # stream layout: for blk: for ch: for q: windows w of blk back-to-back.
    # Each 128-edge tile gets ONE matmul over the 32*k output cols of the
    # k consecutive windows it spans (rhs one-hot col = 32*(win-w0)+slot).
    soff = np.zeros((NWIN, 2, 2), np.int64)    # stream offset of window run
    tbase = np.zeros((NBLK, 2, 2), np.int64)   # global tile base of group
    ntq = np.zeros((NBLK, 2, 2), np.int64)     # tiles in group
    tinfo = []      # per global tile: (w0, k, rco) ; rco = rhs col offset
    w0_of = None
    t = 0
    rcols = 0
    for b in range(NBLK):
        ws = list(range(b * WPB, min((b + 1) * WPB, NWIN)))
        for c in range(2):
            for qq in range(2):
                off = 0
                wspan = {}                     # tile -> [windows]
                for w in ws:
                    soff[w, qq, c] = off
                    cw = int(cap[w, qq, c])
                    if cw == 0:
                        continue
                    for ti in range(off // 128, (off + cw - 1) // 128 + 1):
                        wspan.setdefault(ti, []).append(w)
                    off += cw
                nt = (off + 127) // 128
                tbase[b, qq, c] = t
                ntq[b, qq, c] = nt
                for i in range(nt):
                    wl = wspan.get(i, [ws[0]])
                    w0, k = wl[0], len(wl)
                    tinfo.append((w0, k, rcols))
                    rcols += WIN * k
                t += nt
    nt_total = t

    # per-edge placement (same formula on every core)
    gstart_key = cgid
    order = np.argsort(gstart_key, kind="stable")
    s_inv = np.empty_like(order)
    s_inv[order] = np.arange(len(order))
    counts_flat = np.bincount(gstart_key, minlength=NC * NWIN * 4)
    gstart = np.concatenate([[0], np.cumsum(counts_flat)[:-1]])
    rank = s_inv - gstart[gstart_key]     # rank within (core, win, q, ch)

    blk = win // WPB
    stream_pos = soff[win, q, ch] + rank
    tile_of = tbase[blk, q, ch] + stream_pos // 128
    part_of = stream_pos % 128
    w0_arr = np.array([ti[0] for ti in tinfo], np.int64)
    rco_arr = np.array([ti[2] for ti in tinfo], np.int64)
    col_of = rco_arr[tile_of] + WIN * (win - w0_arr[tile_of]) + dstl - win * WIN

    per_core = []
    for c0 in range(NC):
        m = core == c0
        tiles_c, parts_c = tile_of[m], part_of[m]
        idxflat = np.zeros(nt_total * 128, np.int32)
        relpos = np.where(ch[m] == 0, pos[m], pos[m] - CH0)
        idxflat[tiles_c * 128 + parts_c] = (relpos >> 1)
        assert idxflat.max() < 32768
        idxw = np.zeros((16, nt_total * 8), np.int16)
        fl = np.arange(nt_total * 128)
        idxw[fl % 16, fl // 16] = idxflat.astype(np.int16)
        idxw = np.tile(idxw, (8, 1))                     # [128, nt*8]

        rhs = np.zeros((128, rcols), np.float32)
        rhs[parts_c, col_of[m]] = 1.0
        rhs = rhs.astype(FP8)

        inv_bc = np.broadcast_to(
            invdeg[c0 * NLOC:(c0 + 1) * NLOC], (128, NLOC)).astype(BF16)
        per_core.append((idxw, rhs, np.ascontiguousarray(inv_bc)))

    return dict(tbase=tbase, ntq=ntq, tinfo=tinfo, nt=nt_total, rcols=rcols,
                per_core=per_core, invdeg=invdeg)


def make_host_inputs(h, h_in, weights, W, b, gamma, beta):
    h = np.asarray(h, np.float32)
    h_in = np.asarray(h_in, np.float32)
    # fp8 gather table for h in the chunked position layout
    perm = np.asarray(_pos_of(np.arange(N)))
    inv = np.empty(N, np.int64)
    inv[perm] = np.arange(N)
    h_tab = h[inv].astype(FP8)            # row p = h[node with pos p]
    table_h0 = h_tab[:CH0].reshape(CH0 // 2, 256)
    table_h1 = h_tab[CH0:].reshape(CH1 // 2, 256)

    wT = np.stack([W[m, c].T for m in range(6) for c in range(3)])
    wT = np.ascontiguousarray(
        wT.transpose(1, 0, 2).reshape(128, 18 * 128)).astype(BF16)
    bn = np.zeros((128, 54), np.float32)
    for m in range(6):
        for c in range(3):
            mc = m * 3 + c
            bn[:, 3 * mc + 0] = weights[m, c] * gamma[m, c]
            bn[:, 3 * mc + 1] = weights[m, c] * beta[m, c]
            bn[:, 3 * mc + 2] = b[m, c]
    per_core = []
    for k in range(NC):
        sl = slice(k * NLOC, (k + 1) * NLOC)
        per_core.append(dict(
            hT=np.ascontiguousarray(h[sl].T).astype(BF16),
            hinT=np.ascontiguousarray(h_in[sl].T).astype(BF16),
        ))
    return table_h0, table_h1, wT, bn, per_core


# ---------------------------------------------------------------- device build
def build(struct, stage=99, iters=1):
    import concourse.bass as bass
    import concourse.bacc as bacc
    import concourse.tile as tile
    import concourse.mybir as mybir

    dt = mybir.dt
    AF = mybir.ActivationFunctionType
    OP = mybir.AluOpType
    NT = struct["nt"]
    RCOLS = struct["rcols"]
    tbase, ntq, tinfo = struct["tbase"], struct["ntq"], struct["tinfo"]

    nc = bacc.Bacc("TRN2", target_bir_lowering=False, debug=False)

    th0_in = nc.dram_tensor("table_h0", [CH0 // 2, 256], dt.float8e3,
                            kind="ExternalInput")
    th1_in = nc.dram_tensor("table_h1", [CH1 // 2, 256], dt.float8e3,
                            kind="ExternalInput")
    idxs_in = nc.dram_tensor("idxs", [128, NT * 8], dt.int16, kind="ExternalInput")
    rhs_in = nc.dram_tensor("rhs", [128, RCOLS], dt.float8e3,
                            kind="ExternalInput")
    wT_in = nc.dram_tensor("wT", [128, 18 * 128], dt.float16, kind="ExternalInput")
    bn_in = nc.dram_tensor("bn_small", [128, 54], dt.float32, kind="ExternalInput")
    hT_in = nc.dram_tensor("hT", [128, NLOC], dt.float16, kind="ExternalInput")
    hinT_in = nc.dram_tensor("hinT", [128, NLOC], dt.float16, kind="ExternalInput")
    inv_in = nc.dram_tensor("inv_bc", [128, NLOC], dt.float16, kind="ExternalInput")
    out_cm = nc.dram_tensor("out_cm", [3, 128, NLOC], dt.float16,
                            kind="ExternalOutput")

    ARW_E0 = 129 * 2   # h + h_in moment blocks
    ARW_1 = 129        # single-block allreduces

    with tile.TileContext(nc) as tc:
        import contextlib
        ctx = contextlib.ExitStack()
        with ctx:
            cst = ctx.enter_context(tc.tile_pool(name="cst", bufs=1))
            gat_p = ctx.enter_context(tc.tile_pool(name="gat", bufs=3))
            ztmp_p = ctx.enter_context(tc.tile_pool(name="ztmp", bufs=2))
            nmt_p = ctx.enter_context(tc.tile_pool(name="nmt", bufs=3))
            nm8_p = ctx.enter_context(tc.tile_pool(name="nm8", bufs=2))
            sv_p = ctx.enter_context(tc.tile_pool(name="sv", bufs=2))
            agg_ps = ctx.enter_context(tc.tile_pool(name="aggps", bufs=2, space="PSUM"))
            z_ps = ctx.enter_context(tc.tile_pool(name="zps", bufs=2, space="PSUM"))
            sm_ps = ctx.enter_context(tc.tile_pool(name="smps", bufs=2, space="PSUM"))
            dram = ctx.enter_context(tc.tile_pool(name="dram", bufs=1, space="DRAM"))

            # ---------- resident tiles ----------
            idx_sb = cst.tile([128, NT * 8], dt.int16)
            rhs_sb = cst.tile([128, RCOLS], dt.float8e3)
            wt_sb = cst.tile([128, 18 * 128], dt.float16)
            inv_sb = cst.tile([128, NLOC], dt.float16)
            xt_all = cst.tile([128, 4 * NLOC], dt.float16)
            hT = xt_all[:, 0 * NLOC:1 * NLOC]
            hinT = xt_all[:, 1 * NLOC:2 * NLOC]
            s1T = xt_all[:, 2 * NLOC:3 * NLOC]
            s2T = xt_all[:, 3 * NLOC:4 * NLOC]
            agg_all = cst.tile([128, 3 * NLOC], dt.float16)
            aggT = [agg_all[:, a * NLOC:(a + 1) * NLOC] for a in range(3)]
            acc = cst.tile([128, NLOC], dt.float16)
            # packed bf16 smalls: ident(128) c_bf(7x128) s_bf(7) ones(1)
            sb_bf = cst.tile([128, 128 + 7 * 128 + 7 + 1], dt.float16)
            ident = sb_bf[:, 0:128]
            c_bf = [sb_bf[:, 128 + 128 * s:128 + 128 * (s + 1)] for s in range(7)]
            s_bf = [sb_bf[:, 1024 + s:1025 + s] for s in range(7)]
            ones_bf = sb_bf[:, 1031:1032]
            # packed f32 smalls: bn(54) arstage(258) ar_sb(258) scale(18)
            # bias(18) eps(1)
            W_F32 = 54 + 2 * ARW_E0 + 18 + 18 + 1
            sb_f32 = cst.tile([128, W_F32], dt.float32)
            bn_sb = sb_f32[:, 0:54]
            arstage = sb_f32[:, 54:54 + ARW_E0]
            ar_sb = sb_f32[:, 54 + ARW_E0:54 + 2 * ARW_E0]
            bn_scale = sb_f32[:, 54 + 2 * ARW_E0:54 + 2 * ARW_E0 + 18]
            bn_bias = sb_f32[:, 54 + 2 * ARW_E0 + 18:54 + 2 * ARW_E0 + 36]
            eps_sv = sb_f32[:, 54 + 2 * ARW_E0 + 36:54 + 2 * ARW_E0 + 37]

            ag_in0 = dram.tile([CHL, D], dt.float8e3)
            ag_in1 = dram.tile([NLOC - CHL, D], dt.float8e3)
            ar_in_e0 = dram.tile([128, ARW_E0], dt.float32, name="ar_in_e0")
            ar_ins1 = [dram.tile([128, ARW_1], dt.float32, name=f"ar_in{r}")
                       for r in range(5)]
            cur = {}

            # ---------- prep ----------
            nc.sync.dma_start(idx_sb[:], idxs_in[:])
            nc.sync.dma_start(rhs_sb[:], rhs_in[:])
            nc.sync.dma_start(wt_sb[:], wT_in[:])
            nc.sync.dma_start(bn_sb[:], bn_in[:])
            nc.sync.dma_start(hT[:], hT_in[:])
            nc.sync.dma_start(hinT[:], hinT_in[:])
            nc.sync.dma_start(inv_sb[:], inv_in[:])
            nc.gpsimd.memset(ones_bf[:], 1.0)
            nc.gpsimd.memset(eps_sv[:], EPS)
            from concourse.masks import make_identity
            make_identity(nc, ident[:])

            def gather_raw(out_ap, in_view, c0, ct):
                """dma_gather of ct tiles (128 idx each), 128B fp8 elems at
                256B stride.  Mirrors BassGpSimd.dma_gather minus the
                %256 elem-size assert (stride stays %256 as ucode needs)."""
                g = nc.gpsimd
                n_idx = ct * 128
                idxs_ap = idx_sb[:, 8 * c0:8 * (c0 + ct)]
                _in_ap = g.lower_ap_dma(in_view, for_custom_bir_dma=True)
                inst = g.add_instruction(mybir.InstDMAGatherAnt(
                    name=g.bass.get_next_instruction_name(),
                    ins=[*_in_ap, g.lower_ap(idxs_ap),
                         g.lower_val_access(g.to_reg(n_idx))],
                    outs=[g.lower_ap(out_ap)],
                    transpose=False, num_idxs=n_idx, elem_size=128,
                    stride_bytes_256=1, gen_mode=0, single_packet=False,
                    queue_num=0, sbuf_tokens_per_rank=0,
                    sbuf_free_dim_per_rank=0, sbuf_free_dim_pad_per_rank=0,
                    sbuf_byte_offset=0))
                return inst

            def moment_psums():
                mt = sm_ps.tile([128, 129], dt.float32, space="PSUM",
                                tag="momCS", bufs=1)
                return mt[:, 0:128], mt[:, 128:129]

            def moment_acc(cps, sps, xt, t):
                nc.tensor.matmul(cps, lhsT=xt, rhs=xt,
                                 start=(t == 0), stop=(t == NT_NODES - 1),
                                 skip_group_check=True)
                nc.tensor.matmul(sps, lhsT=xt, rhs=ones_bf[:],
                                 start=False, stop=(t == NT_NODES - 1),
                                 skip_group_check=True)

            def moment_evict(cps, sps, stg, blk):
                co = 129 * blk
                nc.vector.tensor_scalar_mul(stg[:, co:co + 128], cps[:], 1.0 / N)
                nc.vector.tensor_scalar_mul(stg[:, co + 128:co + 129], sps[:],
                                            1.0 / N)

            def moments_from_cm(xT, stg, blk, sink=None):
                """Moments of a ch-major [128, NLOC] state via PE transposes.
                sink(tn, nm_ap, wv) optionally consumes node-major tiles."""
                cps, sps = moment_psums()
                for tn in range(NT_NODES):
                    wv = min(128, NLOC - 128 * tn)
                    tp = sm_ps.tile([128, 128], dt.float16, space="PSUM", tag="tp")
                    nc.tensor.transpose(tp[:wv, :], xT[:, 128 * tn:128 * tn + wv],
                                        ident[:])
                    nm_t = nmt_p.tile([128, 128], dt.float16, tag="nmt")
                    if wv < 128:
                        nc.gpsimd.memset(nm_t[96:, :], 0)
                    nc.scalar.activation(nm_t[:wv, :], tp[:wv, :], AF.Copy,
                                         bias=0.0, scale=1.0)
                    moment_acc(cps, sps, nm_t[:], tn)
                    if sink is not None:
                        sink(tn, nm_t, wv)
                moment_evict(cps, sps, stg, blk)

            # ---------- aggregation ----------
            def agg_phase(a, tbl0, tbl1):
                """agg into aggT[a] from pair-tables (chunk0, chunk1 views).

                Two passes, chunk-outer: pass 0 (chunk-0 edges of every
                block) only needs table chunk 0, so its gathers overlap the
                chunk-1 AllGather still in flight.  Pass 0 evicts invdeg-
                scaled partials into aggT (fp16); pass 1 adds on top."""
                dst = aggT[a]
                views = [
                    (tbl0[:, 0:128], tbl0[:, 128:256]),
                    (tbl1[:, 0:128], tbl1[:, 128:256]),
                ]
                momC, momS = moment_psums()
                tn_all = 0
                for c in range(2):
                    for b in range(NBLK):
                        P = agg_ps.tile([128, 512], dt.float32, space="PSUM",
                                        tag="P")
                        first = True
                        nmm = int(ntq[b, 0, c] + ntq[b, 1, c])
                        done = 0
                        for q in range(2):
                            t0, ntg = int(tbase[b, q, c]), int(ntq[b, q, c])
                            for g0 in range(0, ntg, GCH):
                                ct = min(GCH, ntg - g0)
                                g = gat_p.tile([128, ct, 128], dt.float8e3,
                                               tag="gat")
                                gather_raw(g[:], views[c][q], t0 + g0, ct)
                                for i in range(ct):
                                    t = t0 + g0 + i
                                    w0, k, rco = tinfo[t]
                                    co = WIN * (w0 - b * WPB)
                                    done += 1
                                    nc.tensor.matmul(
                                        P[:, co:co + WIN * k],
                                        lhsT=g[:, i, :],
                                        rhs=rhs_sb[:, rco:rco + WIN * k],
                                        start=first, stop=(done == nmm),
                                        skip_group_check=True)
                                    first = False
                        nbc = min(512, NLOC - 512 * b)
                        dchunk = dst[:, 512 * b:512 * b + nbc]
                        if c == 0:
                            nc.vector.tensor_tensor(
                                out=dchunk, in0=P[:, :nbc],
                                in1=inv_sb[:, 512 * b:512 * b + nbc], op=OP.mult)
                            continue
                        pt = ztmp_p.tile([128, 512], dt.float16, tag="zt")
                        nc.vector.tensor_tensor(
                            out=pt[:, :nbc], in0=P[:, :nbc],
                            in1=inv_sb[:, 512 * b:512 * b + nbc], op=OP.mult)
                        nc.vector.tensor_tensor(
                            out=dchunk, in0=dchunk, in1=pt[:, :nbc], op=OP.add)
                        while tn_all * 128 < 512 * b + nbc:
                            tn = tn_all
                            wv = min(128, NLOC - 128 * tn)
                            tp = sm_ps.tile([128, 128], dt.float16, space="PSUM",
                                            tag="tp")
                            nc.tensor.transpose(tp[:wv, :],
                                                dst[:, 128 * tn:128 * tn + wv],
                                                ident[:])
                            nm_t = nmt_p.tile([128, 128], dt.float16, tag="nmt")
                            if wv < 128:
                                nc.gpsimd.memset(nm_t[96:, :], 0)
                            nc.scalar.activation(nm_t[:wv, :], tp[:wv, :], AF.Copy,
                                                 bias=0.0, scale=1.0)
                            moment_acc(momC, momS, nm_t[:], tn)
                            tn_all += 1
                return momC, momS

            # ---------- collectives ----------
            def fire_ar(ar_in, ar_out, stg_ap):
                nc.sync.dma_start(ar_in[:], stg_ap)
                nc.gpsimd.collective_compute(
                    "AllReduce", OP.add, replica_groups=[list(range(NC))],
                    ins=[ar_in.opt()], outs=[ar_out.opt()])

            def unpack_ar(ar_out, blocks):
                """blocks: list of (block idx in ar_out, state id)"""
                w = 129 * len(blocks)
                nc.sync.dma_start(ar_sb[:, :w], ar_out[:])
                for i, (blk, sid) in enumerate(blocks):
                    co = 129 * i
                    nc.vector.tensor_copy(c_bf[sid][:], ar_sb[:, co:co + 128])
                    nc.vector.tensor_copy(s_bf[sid][:], ar_sb[:, co + 128:co + 129])

            # ---------- BN params for one (m, c) ----------
            def bn_params(mc, sid):
                wt = wt_sb[:, 128 * mc:128 * (mc + 1)]
                wgam = bn_sb[:, 3 * mc + 0:3 * mc + 1]
                wbeta = bn_sb[:, 3 * mc + 1:3 * mc + 2]
                blin = bn_sb[:, 3 * mc + 2:3 * mc + 3]
                vps = sm_ps.tile([128, 128], dt.float32, space="PSUM", tag="tp")
                nc.tensor.matmul(vps[:], lhsT=c_bf[sid][:], rhs=wt, start=True,
                                 stop=True)
                vsb = nmt_p.tile([128, 128], dt.float16, tag="nmt")
                nc.vector.tensor_copy(vsb[:], vps[:])
                msb = nmt_p.tile([128, 128], dt.float16, tag="nmt")
                nc.vector.tensor_tensor(out=msb[:], in0=vsb[:], in1=wt, op=OP.mult)
                bnv = sm_ps.tile([128, 2], dt.float32, space="PSUM", tag="bnv",
                                 bufs=1)
                d2, ws_ = bnv[:, 0:1], bnv[:, 1:2]
                nc.tensor.matmul(d2, lhsT=msb[:], rhs=ones_bf[:], start=True,
                                 stop=True, skip_group_check=True)
                nc.tensor.matmul(ws_, lhsT=wt, rhs=s_bf[sid][:], start=False,
                                 stop=True, skip_group_check=True)
                sv = sv_p.tile([128, 10], dt.float32, tag="sv")
                mu, t1, t2, e2, mu2, var, sd, rv, t4, t5 = (
                    sv[:, i:i + 1] for i in range(10))
                nc.vector.tensor_tensor(out=mu, in0=ws_[:], in1=blin, op=OP.add)
                nc.vector.tensor_scalar(out=t1, in0=ws_[:], scalar1=2.0,
                                        scalar2=blin, op0=OP.mult, op1=OP.add)
                nc.vector.tensor_tensor(out=t2, in0=t1, in1=blin, op=OP.mult)
                nc.vector.tensor_tensor(out=e2, in0=d2[:], in1=t2, op=OP.add)
                nc.vector.tensor_tensor(out=mu2, in0=mu, in1=mu, op=OP.mult)
                nc.vector.tensor_tensor(out=var, in0=e2, in1=mu2, op=OP.subtract)
                nc.scalar.activation(sd, var, AF.Sqrt, bias=eps_sv, scale=1.0)
                nc.vector.reciprocal(rv, sd)
                nc.vector.tensor_tensor(out=bn_scale[:, mc:mc + 1], in0=wgam,
                                        in1=rv, op=OP.mult)
                nc.vector.tensor_tensor(out=t4, in0=blin, in1=mu, op=OP.subtract)
                nc.vector.tensor_tensor(out=t5, in0=bn_scale[:, mc:mc + 1],
                                        in1=t4, op=OP.mult)
                nc.vector.tensor_tensor(out=bn_bias[:, mc:mc + 1], in0=wbeta,
                                        in1=t5, op=OP.add)

            # ---------- z partials ----------
            zstate = {"first": True}

            def z_partial(r, cs):
                for j, m in enumerate(ROUND_MS[r]):
                    for c in cs:
                        mc = m * 3 + c
                        rx = (aggT[j] if c == 0 else
                              [hT, s1T, s2T][j] if c == 1 else hinT)
                        lw = wt_sb[:, 128 * mc:128 * (mc + 1)]
                        off = 0
                        for cw in ZC:
                            zp = z_ps.tile([128, 512], dt.float32, space="PSUM",
                                           tag="zp")
                            nc.tensor.matmul(zp[:, :cw], lhsT=lw,
                                             rhs=rx[:, off:off + cw],
                                             start=True, stop=True)
                            if zstate["first"]:
                                nc.scalar.activation(
                                    acc[:, off:off + cw], zp[:, :cw], AF.Relu,
                                    bias=bn_bias[:, mc:mc + 1],
                                    scale=bn_scale[:, mc:mc + 1])
                            else:
                                zt = ztmp_p.tile([128, 512], dt.float16, tag="zt")
                                nc.scalar.activation(
                                    zt[:, :cw], zp[:, :cw], AF.Relu,
                                    bias=bn_bias[:, mc:mc + 1],
                                    scale=bn_scale[:, mc:mc + 1])
                                nc.vector.tensor_tensor(
                                    out=acc[:, off:off + cw],
                                    in0=acc[:, off:off + cw],
                                    in1=zt[:, :cw], op=OP.add)
                            off += cw
                        zstate["first"] = False

            # ---------- fused round tail: z c0 + out + incremental
            # finalize (sT copy, moments, fp8 cast, chunked AllGather) ----
            def finish_round(r):
                """c0 z-matmuls chunk by chunk; each finished 512-chunk
                immediately streams out_cm, copies into sT, transposes,
                accumulates moments and DMAs fp8 rows; AG0 fires as soon as
                the first CHL rows are on DRAM (r<2 only)."""
                ms = ROUND_MS[r]
                sT = [s1T, s2T][r] if r < 2 else None
                cps = sps = None
                if r < 2:
                    cps, sps = moment_psums()
                tn = 0
                off = 0
                for ci, cw in enumerate(ZC):
                    for j, m in enumerate(ms):
                        mc = m * 3 + 0
                        lw = wt_sb[:, 128 * mc:128 * (mc + 1)]
                        zp = z_ps.tile([128, 512], dt.float32, space="PSUM",
                                       tag="zp")
                        nc.tensor.matmul(zp[:, :cw], lhsT=lw,
                                         rhs=aggT[j][:, off:off + cw],
                                         start=True, stop=True)
                        zt = ztmp_p.tile([128, 512], dt.float16, tag="zt")
                        nc.scalar.activation(zt[:, :cw], zp[:, :cw], AF.Relu,
                                             bias=bn_bias[:, mc:mc + 1],
                                             scale=bn_scale[:, mc:mc + 1])
                        nc.vector.tensor_tensor(
                            out=acc[:, off:off + cw], in0=acc[:, off:off + cw],
                            in1=zt[:, :cw], op=OP.add)
                    nc.sync.dma_start(out_cm[r][:, off:off + cw],
                                      acc[:, off:off + cw])
                    if r < 2:
                        nc.vector.tensor_copy(sT[:, off:off + cw],
                                              acc[:, off:off + cw])
                        while tn * 128 < off + cw:
                            wv = min(128, NLOC - 128 * tn)
                            tp = sm_ps.tile([128, 128], dt.float16,
                                            space="PSUM", tag="tp")
                            nc.tensor.transpose(
                                tp[:wv, :], sT[:, 128 * tn:128 * tn + wv],
                                ident[:])
                            nm_t = nmt_p.tile([128, 128], dt.float16, tag="nmt")
                            if wv < 128:
                                nc.gpsimd.memset(nm_t[96:, :], 0)
                            nc.scalar.activation(nm_t[:wv, :], tp[:wv, :],
                                                 AF.Copy, bias=0.0, scale=1.0)
                            moment_acc(cps, sps, nm_t[:], tn)
                            nm8 = nm8_p.tile([128, 128], dt.float8e3, tag="nm8")
                            nc.vector.tensor_copy(nm8[:wv, :], nm_t[:wv, :])
                            n0 = 128 * tn
                            if n0 + wv <= CHL:
                                nc.gpsimd.dma_start(ag_in0[n0:n0 + wv, :],
                                                    nm8[:wv, :])
                            else:
                                nc.gpsimd.dma_start(
                                    ag_in1[n0 - CHL:n0 - CHL + wv, :],
                                    nm8[:wv, :])
                            tn += 1
                            if tn * 128 == CHL:
                                t0, _ = cur["tbls"][r]
                                nc.gpsimd.collective_compute(
                                    "AllGather", OP.bypass,
                                    replica_groups=[list(range(NC))],
                                    ins=[ag_in0.opt()], outs=[t0.opt()])
                    off += cw
                if r < 2:
                    _, t1_ = cur["tbls"][r]
                    nc.gpsimd.collective_compute(
                        "AllGather", OP.bypass, replica_groups=[list(range(NC))],
                        ins=[ag_in1.opt()], outs=[t1_.opt()])
                    moment_evict(cps, sps, arstage, 0)
                    fire_ar(ar_ins1[2 * r], cur["ar_e"][r], arstage[:, 0:ARW_1])
                    return cur["tbls"][r]

            def agg_and_bn(r, tbl0, tbl1):
                """agg phase r + late AR + c0 bn."""
                momC, momS = agg_phase(r, tbl0, tbl1)
                moment_evict(momC, momS, arstage, 1)
                fire_ar(ar_ins1[2 * r + 1] if r < 2 else ar_ins1[4],
                        cur["ar_l"][r], arstage[:, 129:129 + ARW_1])

            # ================= main schedule =================
            def emit():
                zstate["first"] = True
                moments_from_cm(hT, arstage, 0)
                moments_from_cm(hinT, arstage, 1)
                fire_ar(ar_in_e0, cur["ar_e0"], arstage[:])

                # round 0
                agg_and_bn(0, th0_in[:, :], th1_in[:, :])
                if stage == 1:
                    off = 0
                    for cw in ZC:
                        nc.vector.tensor_copy(acc[:, off:off + cw],
                                              aggT[0][:, off:off + cw])
                        off += cw
                    nc.sync.dma_start(out_cm[0], acc[:])
                    return
                unpack_ar(cur["ar_e0"], [(0, S_H), (1, S_HIN)])
                bn_params(1, S_H)
                bn_params(2, S_HIN)
                z_partial(0, (1, 2))
                unpack_ar(cur["ar_l"][0], [(0, S_A0)])
                bn_params(0, S_A0)
                tbl10, tbl11 = finish_round(0)
                if stage == 3:
                    return
                zstate["first"] = True

                # round 1: identity/skip branches first (under AG+gather)
                unpack_ar(cur["ar_e"][0], [(0, S_S1)])
                for m, sid1 in ((1, S_H), (2, S_S1)):
                    bn_params(m * 3 + 1, sid1)
                    bn_params(m * 3 + 2, S_HIN)
                z_partial(1, (1, 2))
                agg_and_bn(1, tbl10, tbl11)
                unpack_ar(cur["ar_l"][1], [(0, S_A1)])
                bn_params(1 * 3 + 0, S_A0)
                bn_params(2 * 3 + 0, S_A1)
                tbl20, tbl21 = finish_round(1)
                zstate["first"] = True

                # round 2
                unpack_ar(cur["ar_e"][1], [(0, S_S2)])
                for m, sid1 in ((3, S_H), (4, S_S1), (5, S_S2)):
                    bn_params(m * 3 + 1, sid1)
                    bn_params(m * 3 + 2, S_HIN)
                z_partial(2, (1, 2))
                agg_and_bn(2, tbl20, tbl21)
                unpack_ar(cur["ar_l"][2], [(0, S_A2)])
                bn_params(3 * 3 + 0, S_A0)
                bn_params(4 * 3 + 0, S_A1)
                bn_params(5 * 3 + 0, S_A2)
                finish_round(2)

            for it in range(iters):
                cur["tbls"] = [
                    (dram.tile([CH0 // 2, 256], dt.float8e3, addr_space="Shared",
                               name=f"tbl{r}0_{it}"),
                     dram.tile([CH1 // 2, 256], dt.float8e3, addr_space="Shared",
                               name=f"tbl{r}1_{it}"))
                    for r in range(2)]
                cur["ar_e0"] = dram.tile([128, ARW_E0], dt.float32,
                                         addr_space="Shared", name=f"ar_e0_{it}")
                cur["ar_e"] = [dram.tile([128, ARW_1], dt.float32,
                                         addr_space="Shared", name=f"ar_e{r}_{it}")
                               for r in range(2)]
                cur["ar_l"] = [dram.tile([128, ARW_1], dt.float32,
                                         addr_space="Shared", name=f"ar_l{r}_{it}")
                               for r in range(3)]
                emit()

    nc.compile()
    return nc


# ---------------------------------------------------------------- entry point
def prepare(edge_index, h, h_in, weights, W, b, gamma, beta, stage=99, iters=1):
    struct = preprocess(edge_index)
    th0, th1, wT, bn, per_core_s = make_host_inputs(
        h, h_in, weights, W, b, gamma, beta)
    in_maps = []
    for k in range(NC):
        idxw, rhs, inv_bc = struct["per_core"][k]
        in_maps.append(dict(
            table_h0=th0, table_h1=th1, idxs=idxw, rhs=rhs, wT=wT,
            bn_small=bn, inv_bc=inv_bc,
            hT=per_core_s[k]["hT"], hinT=per_core_s[k]["hinT"]))
    nc = build(struct, stage=stage, iters=iters)
    return nc, in_maps


def assemble(results):
    out = np.empty((3, N, D), np.float32)
    for k in range(NC):
        cm = results[k]["out_cm"]
        for r in range(3):
            out[r, k * NLOC:(k + 1) * NLOC, :] = cm[r].T
    return out


def kernel(edge_index, h, h_in, weights, W, b, gamma, beta):
    from concourse.bass_utils import run_bass_kernel_spmd
    nc, in_maps = prepare(np.asarray(edge_index), h, h_in,
                          np.asarray(weights, np.float32),
                          np.asarray(W, np.float32), np.asarray(b, np.float32),
                          np.asarray(gamma, np.float32),
                          np.asarray(beta, np.float32))
    res = run_bass_kernel_spmd(nc, in_maps, core_ids=list(range(NC)))
    return assemble(res.results)
